# revision 43
# baseline (speedup 1.0000x reference)
"""Trainium2 Bass kernel for the APINN-Beam MoE forward pass.

8-core pure data parallel: x [131072,1] sharded along N, weights replicated.
Per-core layout: features on SBUF partitions, N streamed along the free dim.

Structure per 1024-point block (16 blocks/core):
  - shared MLP (1-112-112-112-48) and gate MLP (1-16-16-16-4) fused into one
    chain of block-diagonal matmuls [128,128] (gate rows 0-15, shared 16-127),
    so each tanh covers both networks in one ACT instruction.
  - last fused layer emits [52, Nb]: rows 0-47 feat, 48-51 gate logits.
  - softmax via exp (ACT) + ones-matmul partition sum (PE) + reciprocal (DVE)
    + ones-broadcast matmul (PE) + multiply (DVE).
  - 4 expert MLPs (48-112-112-112-2); their 2-wide heads all accumulate into
    one [8, Nb] PSUM tile via zero-padded [112,8] weights, with the bias
    folded in as a ones-row matmul.
  - u = (sum_e 0.01*w_e*raw0_e) * x * (x-1) via scaled ones-matmul + DVE.
  - raw1 and softmax weights are parked in a resident [8,16384] SBUF tile;
    a second phase applies Softplus (separate ACT table set -> exactly one
    table switch) and the EI combine.
"""
import numpy as np
import ml_dtypes

BF16 = np.float16
_CACHE = {}

N_FULL = 131072
N_CORES = 8
N_SHARD = N_FULL // N_CORES  # 16384
NB = 2048                    # block size (free-dim) per ACT instruction
NBLK = N_SHARD // NB         # 8
MM = 512                     # matmul free-dim slice (one PSUM bank)
L = 1.0


def _split_multiwait(nc):
    """walrus in this toolchain caps sem waits per instruction (2 for compute
    instructions, 1 for CTRL-class like Drain). Split extras into standalone
    NoOps on the same engine, placed just before the offending instruction."""
    import concourse.mybir as mybir

    uid = [0]
    for f in nc.m.functions:
        for bb in f.blocks:
            insts = bb.instructions
            new_list = []
            changed = False
            for inst in insts:
                si = inst.sync_info
                tn = type(inst).__name__
                cap = 1
                if si and si.on_wait and len(si.on_wait) > cap:
                    waits = list(si.on_wait)
                    extra, keep = waits[:-cap], waits[-cap:]
                    for i in range(0, len(extra), 1):
                        uid[0] += 1
                        new_list.append(
                            mybir.InstNoOp(
                                name=f"I-wsplit{uid[0]}",
                                engine=inst.engine,
                                sync_info=mybir.SyncInfo(
                                    on_wait=[extra[i]], on_update=[]
                                ),
                            )
                        )
                    si.on_wait = keep
                    changed = True
                new_list.append(inst)
            if changed:
                bb.instructions = new_list


def _build():
    import concourse.bass as bass
    import concourse.mybir as mybir
    import concourse.tile as tile
    from concourse.tile_rust import add_dep_helper

    f32 = mybir.dt.float32
    AF = mybir.ActivationFunctionType

    bf = mybir.dt.float16
    nc = bass.Bass()
    dp = nc.declare_dram_parameter
    x_d = dp("x", [1, N_SHARD], f32, isOutput=False)
    xb_d = dp("xb", [1, N_SHARD], bf, isOutput=False)
    w1s_d = dp("w1s", [128, 1], f32, isOutput=False)   # layer-1 scale (per partition)
    wc2_d = dp("wc2", [128, 128], bf, isOutput=False)
    wc3_d = dp("wc3", [128, 128], bf, isOutput=False)
    wlg_d = dp("wlg", [128, 4], bf, isOutput=False)    # gate head only
    bc1_d = dp("bc1", [128, 1], f32, isOutput=False)
    bc2_d = dp("bc2", [128, 1], f32, isOutput=False)
    bc3_d = dp("bc3", [128, 1], f32, isOutput=False)
    blg_d = dp("blg", [4, 1], f32, isOutput=False)
    we1_d = dp("we1", [4, 128, 112], bf, isOutput=False)  # composed W4[:, :48] @ We1
    we2_d = dp("we2", [4, 112, 112], bf, isOutput=False)
    we3_d = dp("we3", [4, 112, 112], bf, isOutput=False)
    wf_d = dp("wf", [4, 112, 36], bf, isOutput=False)
    eb1_d = dp("eb1", [4, 112, 1], f32, isOutput=False)
    eb2_d = dp("eb2", [4, 112, 1], f32, isOutput=False)
    eb3_d = dp("eb3", [4, 112, 1], f32, isOutput=False)
    brow_d = dp("brow", [1, 36], bf, isOutput=False)
    out_d = dp("out", [10, N_SHARD], f32, isOutput=True)

    with tile.TileContext(nc) as tc:
        with (
            tc.tile_pool(name="const", bufs=1) as cpool,
            tc.tile_pool(name="work", bufs=2) as wpool,
            tc.tile_pool(name="chain", bufs=3) as hpool,
            tc.tile_pool(name="php", bufs=4, space="PSUM") as php,
            tc.tile_pool(name="praw", bufs=1, space="PSUM") as praw,
            tc.tile_pool(name="psm", bufs=2, space="PSUM") as psm,
        ):
            # ---- constants / weights ----
            def cload(name, dram_ap, shape, dt=f32):
                t = cpool.tile(shape, dt, tag=name, name=name)
                nc.sync.dma_start(t[:], dram_ap)
                return t

            w1s = cload("w1s", w1s_d[:], [128, 1])
            cw2 = cload("cw2", wc2_d[:], [128, 128], bf)
            cw3 = cload("cw3", wc3_d[:], [128, 128], bf)
            cwlg = cload("cwlg", wlg_d[:], [128, 4], bf)
            cb1 = cload("cb1", bc1_d[:], [128, 1])
            cb2 = cload("cb2", bc2_d[:], [128, 1])
            cb3 = cload("cb3", bc3_d[:], [128, 1])
            cblg = cload("cblg", blg_d[:], [4, 1])
            cbrow = cload("cbrow", brow_d[:], [1, 36], bf)
            ew1 = [cload(f"ew1_{e}", we1_d[e], [128, 112], bf) for e in range(4)]
            ew2 = [cload(f"ew2_{e}", we2_d[e], [112, 112], bf) for e in range(4)]
            ew3 = [cload(f"ew3_{e}", we3_d[e], [112, 112], bf) for e in range(4)]
            ewf = [cload(f"ewf_{e}", wf_d[e], [112, 36], bf) for e in range(4)]
            eb1 = [cload(f"eb1_{e}", eb1_d[e], [112, 1]) for e in range(4)]
            eb2 = [cload(f"eb2_{e}", eb2_d[e], [112, 1]) for e in range(4)]
            eb3 = [cload(f"eb3_{e}", eb3_d[e], [112, 1]) for e in range(4)]

            ones_row = cpool.tile([1, MM], bf, tag="ones_row")
            nc.gpsimd.memset(ones_row[:], 1.0)
            ones_c4 = cpool.tile([4, 1], bf, tag="ones_c4")
            nc.gpsimd.memset(ones_c4[:], 1.0)
            cent_c4 = cpool.tile([4, 1], bf, tag="cent_c4")
            nc.gpsimd.memset(cent_c4[:], 0.01)
            ones_r4 = cpool.tile([1, 4], bf, tag="ones_r4")
            nc.gpsimd.memset(ones_r4[:], 1.0)

            # resident: rows 0-3 softmax weights, rows 32-35 exp(raw1)
            res = cpool.tile([36, N_SHARD], f32, tag="res")

            SL = [(s * MM, (s + 1) * MM) for s in range(NB // MM)]

            # ================= phase 1 (software-pipelined) =================
            def front(b):
                c0, c1 = b * NB, (b + 1) * NB
                xt = wpool.tile([1, NB], f32, tag="xt", bufs=2, name=f"xt{b}")
                nc.sync.dma_start(xt[:], x_d[0:1, c0:c1])
                xbc = wpool.tile([128, NB], bf, tag="xbc", name=f"xbc{b}")
                nc.sync.dma_start(xbc[:], xb_d[0:1, c0:c1].broadcast_to([128, NB]))
                h = hpool.tile([128, NB], bf, tag="hs", bufs=4, name=f"h1_{b}")
                nc.scalar.activation(h[:], xbc[:], AF.Tanh, bias=cb1[:], scale=w1s[:])
                for cwi, (cw, cb) in enumerate(((cw2, cb2), (cw3, cb3))):
                    hn = hpool.tile([128, NB], bf, tag="hs", bufs=4, name=f"h{cwi+2}_{b}")
                    for lo, hi in SL:
                        hp = php.tile([128, MM], f32, tag="hp", name=f"hp{b}_{cwi}_{lo}")
                        nc.tensor.matmul(hp[:], cw[:], h[:, lo:hi], start=True, stop=True)
                        nc.scalar.activation(hn[:, lo:hi], hp[:], AF.Tanh, bias=cb[:])
                    h = hn
                xf = wpool.tile([1, NB], bf, tag="xf", bufs=2, name=f"xf{b}")
                nc.vector.tensor_scalar_add(xf[:], xt[:], -L)
                nc.vector.tensor_mul(xf[:], xf[:], xt[:])
                return h, xf

            def tail_a(b, h, xf):
                c0, c1 = b * NB, (b + 1) * NB
                logits32 = wpool.tile([4, NB], f32, tag="lg", bufs=2, name=f"lg{b}")
                for lo, hi in SL:
                    lp = psm.tile([4, MM], f32, tag="sm", name=f"lp{b}_{lo}")
                    nc.tensor.matmul(lp[:], cwlg[:], h[:, lo:hi], start=True, stop=True)
                    nc.vector.tensor_scalar_add(logits32[:, lo:hi], lp[:], cblg[:])
                nc.sync.dma_start(out_d[6:10, c0:c1], logits32[:])
                exps = wpool.tile([4, NB], bf, tag="exps", bufs=2, name=f"ex{b}")
                nc.scalar.activation(exps[:], logits32[:], AF.Exp)
                rcp = wpool.tile([1, NB], bf, tag="rcp", bufs=1, name=f"rcp{b}")
                for lo, hi in SL:
                    ssum = psm.tile([1, MM], f32, tag="sm", name=f"ss{b}_{lo}")
                    nc.tensor.matmul(ssum[:], ones_c4[:], exps[:, lo:hi], start=True, stop=True)
                    with nc.allow_low_precision(reason="softmax denom"):
                        nc.vector.reciprocal(rcp[0:1, lo:hi], ssum[:])
                for lo, hi in SL:
                    wb = psm.tile([4, MM], f32, tag="sm", name=f"wb{b}_{lo}")
                    nc.tensor.matmul(wb[:], ones_r4[:], rcp[0:1, lo:hi], start=True, stop=True)
                    nc.vector.tensor_mul(
                        res[0:4, c0 + lo : c0 + hi], exps[:, lo:hi], wb[:]
                    )
                nc.sync.dma_start(out_d[2:6, c0:c1], res[0:4, c0:c1])

                ehs = {}
                for pair in ((0, 1), (2, 3)):
                    for e in pair:
                        eh = hpool.tile([112, NB], bf, tag="es", bufs=7, name=f"eh1_{b}_{e}")
                        for lo, hi in SL:
                            ep = php.tile([112, MM], f32, tag="hp", name=f"e1p{b}_{e}_{lo}")
                            nc.tensor.matmul(ep[:], ew1[e][:], h[:, lo:hi], start=True, stop=True)
                            nc.scalar.activation(eh[:, lo:hi], ep[:], AF.Tanh, bias=eb1[e][:])
                        ehs[e] = eh
                    for li, (ews, ebs) in enumerate(((ew2, eb2), (ew3, eb3))):
                        for e in pair:
                            ehn = hpool.tile([112, NB], bf, tag="es", bufs=7, name=f"eh{li+2}_{b}_{e}")
                            for lo, hi in SL:
                                ep = php.tile([112, MM], f32, tag="hp", name=f"ep{b}_{e}_{li}_{lo}")
                                nc.tensor.matmul(ep[:], ews[e][:], ehs[e][:, lo:hi], start=True, stop=True)
                                nc.scalar.activation(ehn[:, lo:hi], ep[:], AF.Tanh, bias=ebs[e][:])
                            ehs[e] = ehn
                return ehs

            def tail_b(b, h, xf, ehs):
                c0, c1 = b * NB, (b + 1) * NB
                # heads (slice-major), raw1 -> exp -> resident; u combine
                t0 = wpool.tile([4, NB], bf, tag="t0", bufs=2, name=f"t0_{b}")
                ut = wpool.tile([1, NB], f32, tag="ut", bufs=1, name=f"ut{b}")
                lp1 = None
                for s2 in range(len(SL) // 2):
                    o2 = s2 * 2 * MM
                    rp = praw.tile([36, 2 * MM], f32, tag="raw", name=f"rp{b}_{s2}")
                    for half in range(2):
                        nc.tensor.matmul(
                            rp[:, half * MM : (half + 1) * MM],
                            cbrow[:], ones_row[:], start=True, stop=False,
                        )
                    for e in range(4):
                        for half in range(2):
                            lo = o2 + half * MM
                            nc.tensor.matmul(
                                rp[:, half * MM : (half + 1) * MM],
                                ewf[e][:], ehs[e][:, lo : lo + MM],
                                start=False, stop=(e == 3),
                            )
                    lp1 = nc.scalar.activation(
                        res[32:36, c0 + o2 : c0 + o2 + 2 * MM], rp[32:36, :], AF.Exp
                    )
                    nc.vector.tensor_mul(
                        t0[:, o2 : o2 + 2 * MM], rp[0:4, :],
                        res[0:4, c0 + o2 : c0 + o2 + 2 * MM],
                    )
                    for half in range(2):
                        lo = o2 + half * MM
                        u0 = psm.tile([1, MM], f32, tag="sm", name=f"u0{b}_{lo}")
                        nc.tensor.matmul(
                            u0[:], cent_c4[:], t0[:, lo : lo + MM],
                            start=True, stop=True,
                        )
                        nc.vector.tensor_mul(
                            ut[0:1, lo : lo + MM], u0[:], xf[0:1, lo : lo + MM]
                        )
                nc.sync.dma_start(out_d[0:1, c0:c1], ut[:])
                return lp1

            pending = None
            for b in range(NBLK):
                if pending is not None:
                    ehs_p = tail_a(pending[0], pending[1], pending[2])
                fr = front(b)
                if pending is not None:
                    last_p1_act = tail_b(pending[0], pending[1], pending[2], ehs_p)
                pending = (b, fr[0], fr[1])
            ehs_p = tail_a(pending[0], pending[1], pending[2])
            last_p1_act = tail_b(pending[0], pending[1], pending[2], ehs_p)

            # ================= phase 2 (softplus + EI) =================
            for b in range(NBLK):
                c0, c1 = b * NB, (b + 1) * NB
                sp = wpool.tile([4, NB], bf, tag="sp", bufs=1)
                ln_inst = nc.scalar.activation(
                    sp[:], res[32:36, c0:c1], AF.Ln, bias=1.0
                )
                add_dep_helper(ln_inst.ins, last_p1_act.ins, sync=True)
                t1 = wpool.tile([4, NB], bf, tag="t1", bufs=1)
                nc.vector.tensor_mul(t1[:], sp[:], res[0:4, c0:c1])
                ei = wpool.tile([1, NB], f32, tag="ei", bufs=1)
                for lo, hi in SL:
                    e0 = psm.tile([1, MM], f32, tag="sm", name=f"e0{b}_{lo}")
                    nc.tensor.matmul(e0[:], ones_c4[:], t1[:, lo:hi], start=True, stop=True)
                    nc.vector.tensor_copy(ei[0:1, lo:hi], e0[:])
                nc.sync.dma_start(out_d[1:2, c0:c1], ei[:])

    _split_multiwait(nc)
    return nc


def _prep_inputs(x, shared_params, expert_params, gate_params):
    a = lambda v: np.asarray(v, dtype=np.float64)
    sh = [(a(W), a(b)) for W, b in shared_params]
    ga = [(a(W), a(b)) for W, b in gate_params]
    ex = [(a(W), a(b)) for W, b in expert_params]

    # layer 1 (1->128): per-partition scale/bias for the ACT trick
    w1s = np.zeros((128, 1), np.float32)
    w1s[:16, 0] = ga[0][0][0]
    w1s[16:, 0] = sh[0][0][0]
    bc1 = np.zeros((128, 1), np.float32)
    bc1[:16, 0] = ga[0][1]
    bc1[16:, 0] = sh[0][1]

    def blockdiag(gW, sW):
        w = np.zeros((128, 128), np.float64)
        w[:16, :16] = gW
        w[16:, 16:] = sW
        return w

    wc2 = blockdiag(ga[1][0], sh[1][0])
    bc2 = np.concatenate([ga[1][1], sh[1][1]]).reshape(128, 1).astype(np.float32)
    wc3 = blockdiag(ga[2][0], sh[2][0])
    bc3 = np.concatenate([ga[2][1], sh[2][1]]).reshape(128, 1).astype(np.float32)

    # gate head [128, 4] (gate rows only)
    wlg = np.zeros((128, 4), np.float64)
    wlg[:16, :] = ga[3][0]
    blg = ga[3][1].reshape(4, 1).astype(np.float32)

    # shared L4 composed with expert L1: Wcomp_e = Wc4full[:, :48] @ We1_e
    wc4full = np.zeros((128, 48), np.float64)
    wc4full[16:, :] = sh[3][0]
    we1c = np.zeros((4, 128, 112), np.float64)
    eb1c = np.zeros((4, 112, 1), np.float32)
    for e in range(4):
        we1c[e] = wc4full @ ex[0][0][e]
        eb1c[e, :, 0] = (sh[3][1] @ ex[0][0][e] + ex[0][1][e]).astype(np.float32)

    wf = np.zeros((4, 112, 36), np.float64)
    brow = np.zeros((1, 36), np.float64)
    for e in range(4):
        wf[e, :, e] = ex[3][0][e][:, 0]
        wf[e, :, 32 + e] = ex[3][0][e][:, 1]
        brow[0, e] = ex[3][1][e][0]
        brow[0, 32 + e] = ex[3][1][e][1]

    common = {
        "w1s": w1s, "wc2": wc2.astype(BF16), "wc3": wc3.astype(BF16),
        "wlg": wlg.astype(BF16),
        "bc1": bc1, "bc2": bc2, "bc3": bc3, "blg": blg,
        "we1": we1c.astype(BF16), "we2": ex[1][0].astype(BF16),
        "we3": ex[2][0].astype(BF16), "wf": wf.astype(BF16),
        "eb1": eb1c,
        "eb2": ex[1][1].reshape(4, 112, 1).astype(np.float32),
        "eb3": ex[2][1].reshape(4, 112, 1).astype(np.float32),
        "brow": brow.astype(BF16),
    }
    xs = np.asarray(x, np.float32).reshape(N_FULL)
    in_maps = []
    for i in range(N_CORES):
        m = dict(common)
        sh_ = xs[i * N_SHARD : (i + 1) * N_SHARD].reshape(1, N_SHARD)
        m["x"] = sh_.copy()
        m["xb"] = sh_.astype(BF16)
        in_maps.append(m)
    return in_maps


def _run(in_maps, trace=False, **kw):
    from concourse.bass_utils import run_bass_kernel_spmd

    if "nc" not in _CACHE:
        _CACHE["nc"] = _build()
    return run_bass_kernel_spmd(
        _CACHE["nc"], in_maps, list(range(N_CORES)), trace=trace, **kw
    )


def _gather(res):
    outs = [res.results[i]["out"] for i in range(N_CORES)]
    full = np.concatenate(outs, axis=1)  # [10, N_FULL]
    u = full[0].reshape(N_FULL, 1).astype(np.float32)
    ei = full[1].reshape(N_FULL, 1).astype(np.float32)
    w = np.ascontiguousarray(full[2:6].T).astype(np.float32)
    gl = np.ascontiguousarray(full[6:10].T).astype(np.float32)
    return (u, ei, w, gl)


def kernel(x, shared_params, expert_params, gate_params):
    in_maps = _prep_inputs(x, shared_params, expert_params, gate_params)
    res = _run(in_maps, trace=False)
    return _gather(res)


# revision 48
# speedup vs baseline: 1.0523x; 1.0523x over previous
"""Trainium2 Bass kernel for the APINN-Beam MoE forward pass.

8-core pure data parallel: x [131072,1] sharded along N, weights replicated.
Per-core layout: features on SBUF partitions, N streamed along the free dim.

Structure per 1024-point block (16 blocks/core):
  - shared MLP (1-112-112-112-48) and gate MLP (1-16-16-16-4) fused into one
    chain of block-diagonal matmuls [128,128] (gate rows 0-15, shared 16-127),
    so each tanh covers both networks in one ACT instruction.
  - last fused layer emits [52, Nb]: rows 0-47 feat, 48-51 gate logits.
  - softmax via exp (ACT) + ones-matmul partition sum (PE) + reciprocal (DVE)
    + ones-broadcast matmul (PE) + multiply (DVE).
  - 4 expert MLPs (48-112-112-112-2); their 2-wide heads all accumulate into
    one [8, Nb] PSUM tile via zero-padded [112,8] weights, with the bias
    folded in as a ones-row matmul.
  - u = (sum_e 0.01*w_e*raw0_e) * x * (x-1) via scaled ones-matmul + DVE.
  - raw1 and softmax weights are parked in a resident [8,16384] SBUF tile;
    a second phase applies Softplus (separate ACT table set -> exactly one
    table switch) and the EI combine.
"""
import numpy as np
import ml_dtypes

BF16 = np.float16
_CACHE = {}

N_FULL = 131072
N_CORES = 8
N_SHARD = N_FULL // N_CORES  # 16384
NB = 2048                    # block size (free-dim) per ACT instruction
NBLK = N_SHARD // NB         # 8
MM = 512                     # matmul free-dim slice (one PSUM bank)
L = 1.0


def _split_multiwait(nc):
    """walrus in this toolchain caps sem waits per instruction (2 for compute
    instructions, 1 for CTRL-class like Drain). Split extras into standalone
    NoOps on the same engine, placed just before the offending instruction."""
    import concourse.mybir as mybir

    uid = [0]
    for f in nc.m.functions:
        for bb in f.blocks:
            insts = bb.instructions
            new_list = []
            changed = False
            for inst in insts:
                si = inst.sync_info
                tn = type(inst).__name__
                cap = 1
                if si and si.on_wait and len(si.on_wait) > cap:
                    waits = list(si.on_wait)
                    extra, keep = waits[:-cap], waits[-cap:]
                    for i in range(0, len(extra), 1):
                        uid[0] += 1
                        new_list.append(
                            mybir.InstNoOp(
                                name=f"I-wsplit{uid[0]}",
                                engine=inst.engine,
                                sync_info=mybir.SyncInfo(
                                    on_wait=[extra[i]], on_update=[]
                                ),
                            )
                        )
                    si.on_wait = keep
                    changed = True
                new_list.append(inst)
            if changed:
                bb.instructions = new_list


def _build():
    import concourse.bass as bass
    import concourse.mybir as mybir
    import concourse.tile as tile
    from concourse.tile_rust import add_dep_helper

    f32 = mybir.dt.float32
    AF = mybir.ActivationFunctionType

    bf = mybir.dt.float16
    nc = bass.Bass()
    dp = nc.declare_dram_parameter
    x_d = dp("x", [1, N_SHARD], f32, isOutput=False)
    xb_d = dp("xb", [1, N_SHARD], bf, isOutput=False)
    w1s_d = dp("w1s", [128, 1], f32, isOutput=False)   # layer-1 scale (per partition)
    wc2_d = dp("wc2", [128, 128], bf, isOutput=False)
    wc3_d = dp("wc3", [128, 128], bf, isOutput=False)
    wlg_d = dp("wlg", [128, 4], bf, isOutput=False)    # gate head only
    bc1_d = dp("bc1", [128, 1], f32, isOutput=False)
    bc2_d = dp("bc2", [128, 1], f32, isOutput=False)
    bc3_d = dp("bc3", [128, 1], f32, isOutput=False)
    blg_d = dp("blg", [4, 1], f32, isOutput=False)
    we1_d = dp("we1", [4, 128, 112], bf, isOutput=False)  # composed W4[:, :48] @ We1
    we2_d = dp("we2", [4, 112, 112], bf, isOutput=False)
    we3_d = dp("we3", [4, 112, 112], bf, isOutput=False)
    wf_d = dp("wf", [4, 112, 36], bf, isOutput=False)
    eb1_d = dp("eb1", [4, 112, 1], f32, isOutput=False)
    eb2_d = dp("eb2", [4, 112, 1], f32, isOutput=False)
    eb3_d = dp("eb3", [4, 112, 1], f32, isOutput=False)
    brow_d = dp("brow", [1, 36], bf, isOutput=False)
    out_d = dp("out", [10, N_SHARD], f32, isOutput=True)

    with tile.TileContext(nc) as tc:
        with (
            tc.tile_pool(name="const", bufs=1) as cpool,
            tc.tile_pool(name="work", bufs=2) as wpool,
            tc.tile_pool(name="chain", bufs=3) as hpool,
            tc.tile_pool(name="php", bufs=4, space="PSUM") as php,
            tc.tile_pool(name="praw", bufs=1, space="PSUM") as praw,
            tc.tile_pool(name="psm", bufs=2, space="PSUM") as psm,
        ):
            # ---- prefetch block-0 inputs ahead of the weight loads ----
            xt0 = wpool.tile([1, NB], f32, tag="xt", bufs=2, name="xt_pre")
            nc.sync.dma_start(xt0[:], x_d[0:1, 0:NB])
            xbc0 = wpool.tile([128, NB], bf, tag="xbc", name="xbc_pre")
            nc.sync.dma_start(xbc0[:], xb_d[0:1, 0:NB].broadcast_to([128, NB]))

            # ---- constants / weights ----
            def cload(name, dram_ap, shape, dt=f32):
                t = cpool.tile(shape, dt, tag=name, name=name)
                nc.sync.dma_start(t[:], dram_ap)
                return t

            w1s = cload("w1s", w1s_d[:], [128, 1])
            cw2 = cload("cw2", wc2_d[:], [128, 128], bf)
            cw3 = cload("cw3", wc3_d[:], [128, 128], bf)
            cwlg = cload("cwlg", wlg_d[:], [128, 4], bf)
            cb1 = cload("cb1", bc1_d[:], [128, 1])
            cb2 = cload("cb2", bc2_d[:], [128, 1])
            cb3 = cload("cb3", bc3_d[:], [128, 1])
            cblg = cload("cblg", blg_d[:], [4, 1])
            cbrow = cload("cbrow", brow_d[:], [1, 36], bf)
            ew1 = [cload(f"ew1_{e}", we1_d[e], [128, 112], bf) for e in range(4)]
            ew2 = [cload(f"ew2_{e}", we2_d[e], [112, 112], bf) for e in range(4)]
            ew3 = [cload(f"ew3_{e}", we3_d[e], [112, 112], bf) for e in range(4)]
            ewf = [cload(f"ewf_{e}", wf_d[e], [112, 36], bf) for e in range(4)]
            eb1 = [cload(f"eb1_{e}", eb1_d[e], [112, 1]) for e in range(4)]
            eb2 = [cload(f"eb2_{e}", eb2_d[e], [112, 1]) for e in range(4)]
            eb3 = [cload(f"eb3_{e}", eb3_d[e], [112, 1]) for e in range(4)]

            ones_row = cpool.tile([1, MM], bf, tag="ones_row")
            nc.gpsimd.memset(ones_row[:], 1.0)
            ones_c4 = cpool.tile([4, 1], bf, tag="ones_c4")
            nc.gpsimd.memset(ones_c4[:], 1.0)
            cent_c4 = cpool.tile([4, 1], bf, tag="cent_c4")
            nc.gpsimd.memset(cent_c4[:], 0.01)
            ones_r4 = cpool.tile([1, 4], bf, tag="ones_r4")
            nc.gpsimd.memset(ones_r4[:], 1.0)

            # resident: rows 0-3 softmax weights, rows 32-35 exp(raw1)
            res = cpool.tile([36, N_SHARD], f32, tag="res")

            SL = [(s * MM, (s + 1) * MM) for s in range(NB // MM)]

            # ================= phase 1 (software-pipelined) =================
            def front(b):
                c0, c1 = b * NB, (b + 1) * NB
                if b == 0:
                    xt, xbc = xt0, xbc0
                else:
                    xt = wpool.tile([1, NB], f32, tag="xt", bufs=2, name=f"xt{b}")
                    nc.sync.dma_start(xt[:], x_d[0:1, c0:c1])
                    xbc = wpool.tile([128, NB], bf, tag="xbc", name=f"xbc{b}")
                    nc.sync.dma_start(xbc[:], xb_d[0:1, c0:c1].broadcast_to([128, NB]))
                h = hpool.tile([128, NB], bf, tag="hs", bufs=4, name=f"h1_{b}")
                nc.scalar.activation(h[:], xbc[:], AF.Tanh, bias=cb1[:], scale=w1s[:])
                for cwi, (cw, cb) in enumerate(((cw2, cb2), (cw3, cb3))):
                    hn = hpool.tile([128, NB], bf, tag="hs", bufs=4, name=f"h{cwi+2}_{b}")
                    for lo, hi in SL:
                        hp = php.tile([128, MM], f32, tag="hp", name=f"hp{b}_{cwi}_{lo}")
                        nc.tensor.matmul(hp[:], cw[:], h[:, lo:hi], start=True, stop=True)
                        nc.scalar.activation(hn[:, lo:hi], hp[:], AF.Tanh, bias=cb[:])
                    h = hn
                xf = wpool.tile([1, NB], bf, tag="xf", bufs=2, name=f"xf{b}")
                nc.vector.tensor_scalar_add(xf[:], xt[:], -L)
                nc.vector.tensor_mul(xf[:], xf[:], xt[:])
                return h, xf

            def tail_a(b, h, xf):
                c0, c1 = b * NB, (b + 1) * NB
                logits32 = wpool.tile([4, NB], f32, tag="lg", bufs=2, name=f"lg{b}")
                for lo, hi in SL:
                    lp = psm.tile([4, MM], f32, tag="sm", name=f"lp{b}_{lo}")
                    nc.tensor.matmul(lp[:], cwlg[:], h[:, lo:hi], start=True, stop=True)
                    nc.vector.tensor_scalar_add(logits32[:, lo:hi], lp[:], cblg[:])
                nc.sync.dma_start(out_d[6:10, c0:c1], logits32[:])
                exps = wpool.tile([4, NB], bf, tag="exps", bufs=2, name=f"ex{b}")
                exp_inst[0] = nc.scalar.activation(exps[:], logits32[:], AF.Exp)
                rcp = wpool.tile([1, NB], bf, tag="rcp", bufs=1, name=f"rcp{b}")
                for lo, hi in SL:
                    ssum = psm.tile([1, MM], f32, tag="sm", name=f"ss{b}_{lo}")
                    nc.tensor.matmul(ssum[:], ones_c4[:], exps[:, lo:hi], start=True, stop=True)
                    with nc.allow_low_precision(reason="softmax denom"):
                        nc.vector.reciprocal(rcp[0:1, lo:hi], ssum[:])
                for lo, hi in SL:
                    wb = psm.tile([4, MM], f32, tag="sm", name=f"wb{b}_{lo}")
                    nc.tensor.matmul(wb[:], ones_r4[:], rcp[0:1, lo:hi], start=True, stop=True)
                    nc.vector.tensor_mul(
                        res[0:4, c0 + lo : c0 + hi], exps[:, lo:hi], wb[:]
                    )
                nc.sync.dma_start(out_d[2:6, c0:c1], res[0:4, c0:c1])

                ehs = {}
                for pair in ((0, 1), (2, 3)):
                    for e in pair:
                        eh = hpool.tile([112, NB], bf, tag="es", bufs=7, name=f"eh1_{b}_{e}")
                        for lo, hi in SL:
                            ep = php.tile([112, MM], f32, tag="hp", name=f"e1p{b}_{e}_{lo}")
                            nc.tensor.matmul(ep[:], ew1[e][:], h[:, lo:hi], start=True, stop=True)
                            nc.scalar.activation(eh[:, lo:hi], ep[:], AF.Tanh, bias=eb1[e][:])
                        ehs[e] = eh
                    for li, (ews, ebs) in enumerate(((ew2, eb2), (ew3, eb3))):
                        for e in pair:
                            ehn = hpool.tile([112, NB], bf, tag="es", bufs=7, name=f"eh{li+2}_{b}_{e}")
                            for lo, hi in SL:
                                ep = php.tile([112, MM], f32, tag="hp", name=f"ep{b}_{e}_{li}_{lo}")
                                nc.tensor.matmul(ep[:], ews[e][:], ehs[e][:, lo:hi], start=True, stop=True)
                                nc.scalar.activation(ehn[:, lo:hi], ep[:], AF.Tanh, bias=ebs[e][:])
                            ehs[e] = ehn
                return ehs

            def tail_b(b, h, xf, ehs):
                c0, c1 = b * NB, (b + 1) * NB
                # heads (slice-major), raw1 -> exp -> resident; u combine
                t0 = wpool.tile([4, NB], bf, tag="t0", bufs=2, name=f"t0_{b}")
                ut = wpool.tile([1, NB], f32, tag="ut", bufs=1, name=f"ut{b}")
                lp1 = None
                for s2 in range(len(SL) // 2):
                    o2 = s2 * 2 * MM
                    rp = praw.tile([36, 2 * MM], f32, tag="raw", name=f"rp{b}_{s2}")
                    for half in range(2):
                        nc.tensor.matmul(
                            rp[:, half * MM : (half + 1) * MM],
                            cbrow[:], ones_row[:], start=True, stop=False,
                        )
                    for e in range(4):
                        for half in range(2):
                            lo = o2 + half * MM
                            nc.tensor.matmul(
                                rp[:, half * MM : (half + 1) * MM],
                                ewf[e][:], ehs[e][:, lo : lo + MM],
                                start=False, stop=(e == 3),
                            )
                    lp1 = nc.vector.tensor_copy(
                        res[32:36, c0 + o2 : c0 + o2 + 2 * MM], rp[32:36, :]
                    )
                    nc.vector.tensor_mul(
                        t0[:, o2 : o2 + 2 * MM], rp[0:4, :],
                        res[0:4, c0 + o2 : c0 + o2 + 2 * MM],
                    )
                    for half in range(2):
                        lo = o2 + half * MM
                        u0 = psm.tile([1, MM], f32, tag="sm", name=f"u0{b}_{lo}")
                        nc.tensor.matmul(
                            u0[:], cent_c4[:], t0[:, lo : lo + MM],
                            start=True, stop=True,
                        )
                        nc.vector.tensor_mul(
                            ut[0:1, lo : lo + MM], u0[:], xf[0:1, lo : lo + MM]
                        )
                nc.sync.dma_start(out_d[0:1, c0:c1], ut[:])
                return lp1

            exp_inst = [None]
            for b in range(NBLK):
                h_b, xf_b = front(b)
                ehs_b = tail_a(b, h_b, xf_b)
                tail_b(b, h_b, xf_b, ehs_b)
            last_p1_act = exp_inst[0]

            # ================= phase 2 (softplus + EI) =================
            for b in range(NBLK):
                c0, c1 = b * NB, (b + 1) * NB
                spx = wpool.tile([4, NB], bf, tag="spx", bufs=1)
                ex_inst = nc.scalar.activation(spx[:], res[32:36, c0:c1], AF.Exp)
                # keep all phase-2 ACT after phase-1 ACT: one table-set switch
                add_dep_helper(ex_inst.ins, last_p1_act.ins, sync=True)
                sp = wpool.tile([4, NB], bf, tag="sp", bufs=1)
                nc.scalar.activation(sp[:], spx[:], AF.Ln, bias=1.0)
                t1 = wpool.tile([4, NB], bf, tag="t1", bufs=1)
                nc.vector.tensor_mul(t1[:], sp[:], res[0:4, c0:c1])
                ei = wpool.tile([1, NB], f32, tag="ut", bufs=1, name=f"ei{b}")
                for lo, hi in SL:
                    e0 = psm.tile([1, MM], f32, tag="sm", name=f"e0{b}_{lo}")
                    nc.tensor.matmul(e0[:], ones_c4[:], t1[:, lo:hi], start=True, stop=True)
                    nc.vector.tensor_copy(ei[0:1, lo:hi], e0[:])
                nc.sync.dma_start(out_d[1:2, c0:c1], ei[:])

    _split_multiwait(nc)
    return nc


def _prep_inputs(x, shared_params, expert_params, gate_params):
    a = lambda v: np.asarray(v, dtype=np.float64)
    sh = [(a(W), a(b)) for W, b in shared_params]
    ga = [(a(W), a(b)) for W, b in gate_params]
    ex = [(a(W), a(b)) for W, b in expert_params]

    # layer 1 (1->128): per-partition scale/bias for the ACT trick
    w1s = np.zeros((128, 1), np.float32)
    w1s[:16, 0] = ga[0][0][0]
    w1s[16:, 0] = sh[0][0][0]
    bc1 = np.zeros((128, 1), np.float32)
    bc1[:16, 0] = ga[0][1]
    bc1[16:, 0] = sh[0][1]

    def blockdiag(gW, sW):
        w = np.zeros((128, 128), np.float64)
        w[:16, :16] = gW
        w[16:, 16:] = sW
        return w

    wc2 = blockdiag(ga[1][0], sh[1][0])
    bc2 = np.concatenate([ga[1][1], sh[1][1]]).reshape(128, 1).astype(np.float32)
    wc3 = blockdiag(ga[2][0], sh[2][0])
    bc3 = np.concatenate([ga[2][1], sh[2][1]]).reshape(128, 1).astype(np.float32)

    # gate head [128, 4] (gate rows only)
    wlg = np.zeros((128, 4), np.float64)
    wlg[:16, :] = ga[3][0]
    blg = ga[3][1].reshape(4, 1).astype(np.float32)

    # shared L4 composed with expert L1: Wcomp_e = Wc4full[:, :48] @ We1_e
    wc4full = np.zeros((128, 48), np.float64)
    wc4full[16:, :] = sh[3][0]
    we1c = np.zeros((4, 128, 112), np.float64)
    eb1c = np.zeros((4, 112, 1), np.float32)
    for e in range(4):
        we1c[e] = wc4full @ ex[0][0][e]
        eb1c[e, :, 0] = (sh[3][1] @ ex[0][0][e] + ex[0][1][e]).astype(np.float32)

    wf = np.zeros((4, 112, 36), np.float64)
    brow = np.zeros((1, 36), np.float64)
    for e in range(4):
        wf[e, :, e] = ex[3][0][e][:, 0]
        wf[e, :, 32 + e] = ex[3][0][e][:, 1]
        brow[0, e] = ex[3][1][e][0]
        brow[0, 32 + e] = ex[3][1][e][1]

    common = {
        "w1s": w1s, "wc2": wc2.astype(BF16), "wc3": wc3.astype(BF16),
        "wlg": wlg.astype(BF16),
        "bc1": bc1, "bc2": bc2, "bc3": bc3, "blg": blg,
        "we1": we1c.astype(BF16), "we2": ex[1][0].astype(BF16),
        "we3": ex[2][0].astype(BF16), "wf": wf.astype(BF16),
        "eb1": eb1c,
        "eb2": ex[1][1].reshape(4, 112, 1).astype(np.float32),
        "eb3": ex[2][1].reshape(4, 112, 1).astype(np.float32),
        "brow": brow.astype(BF16),
    }
    xs = np.asarray(x, np.float32).reshape(N_FULL)
    in_maps = []
    for i in range(N_CORES):
        m = dict(common)
        sh_ = xs[i * N_SHARD : (i + 1) * N_SHARD].reshape(1, N_SHARD)
        m["x"] = sh_.copy()
        m["xb"] = sh_.astype(BF16)
        in_maps.append(m)
    return in_maps


def _run(in_maps, trace=False, **kw):
    from concourse.bass_utils import run_bass_kernel_spmd

    if "nc" not in _CACHE:
        _CACHE["nc"] = _build()
    return run_bass_kernel_spmd(
        _CACHE["nc"], in_maps, list(range(N_CORES)), trace=trace, **kw
    )


def _gather(res):
    outs = [res.results[i]["out"] for i in range(N_CORES)]
    full = np.concatenate(outs, axis=1)  # [10, N_FULL]
    u = full[0].reshape(N_FULL, 1).astype(np.float32)
    ei = full[1].reshape(N_FULL, 1).astype(np.float32)
    w = np.ascontiguousarray(full[2:6].T).astype(np.float32)
    gl = np.ascontiguousarray(full[6:10].T).astype(np.float32)
    return (u, ei, w, gl)


def kernel(x, shared_params, expert_params, gate_params):
    in_maps = _prep_inputs(x, shared_params, expert_params, gate_params)
    res = _run(in_maps, trace=False)
    return _gather(res)
